# revision 61
# baseline (speedup 1.0000x reference)
"""MoE (top-2 of 8 experts, SwiGLU FFN) on 8 Trainium2 NeuronCores.

Strategy: expert-parallel with fp8 DoubleRow matmuls. Routing (gate matmul +
top-2 + softmax) is done on the host; tokens are gathered per expert, padded
to a common capacity C, and each core runs the full SwiGLU FFN for one
expert's tokens. All three matmuls run on the PE in fp8-e4m3 DoubleRow mode
(2 contraction rows per PE cell) using a 3-term split-precision scheme:

    W ~= Whi + Wlo,  x ~= xhi + xlo   (each term an e4m3 tensor)
    W @ x ~= Whi@xhi + Whi@xlo + Wlo@xhi     (lo@lo dropped, ~delta^2)

which contracts 3x1024 rows per logical 1024-row matmul but at 512 rows per
PE cycle (vs 128 for bf16), i.e. 0.75x the bf16 cycle count with ~2e-3 final
relative error. The hi/lo pairs are laid out so that each DoubleRow matmul
consumes either (main) the hi slots of two adjacent 128-row subtiles or
(cross) the (hi,lo)x(lo,hi) slot pair of one subtile.

All quantization scales are powers of two and are folded into the on-device
activation path (exactly) and into the host-side combine weights.

Device layouts (per core, pre-packed on host):
  xt{i} [128, 2, KD, w]  fp8  per-chunk x: slot0=lo, slot1=hi; d = k*128+p
  w0t/w1t [128, KH, KD, 2, 128] fp8  slot0=hi, slot1=lo; [p=d_row, ht, ks, slot, hcol]
  w2t [128, KD, KH, 2, 128] fp8 slot0=hi, slot1=lo; [p=h_row, dk, ht, slot, dcol]
  b0t/b1t [128, KH]      fp32 per-partition biases (pre-scaled)
  out [128, KD, C]       fp32 transposed: out[p, k, c] = ffn_out[c, k*128+p] * SW*SACT
"""

import os

import numpy as np
import ml_dtypes

# The tunneled trn2 cores occasionally come up wedged from a prior process;
# asking the runtime to reset cores on init recovers them.
os.environ.setdefault("NEURON_RT_RESET_CORES", "1")

E, TOPK, D, H = 8, 2, 1024, 2048
NCORES = 8
P = 128
KD = D // P   # 8 d-tiles
KH = H // P   # 16 h-tiles
F8 = ml_dtypes.float8_e4m3

# power-of-two quantization scales
SX = 16.0      # x * SX fits e4m3 (|x| max ~5.3 -> 85)
SW = 1024.0    # w * SW fits e4m3 (|w| max ~0.11 -> 115)
SACT = 8.0     # act * SACT fits e4m3 (|act| max ~9 -> 72)
B1SC = 1.0 / (SX * SW)     # psum1 -> h1 descale for silu input
B0SC = SACT / (SX * SW)    # psum0*sil -> stored-act scale

_build_cache: dict = {}
_ACT_SILU = True  # CoreSim lacks Silu; tests may flip this to Tanh


def _chunk_plan(C: int):
    """Token-chunk widths (DoubleRow caps the moving dim at 256).

    Largest chunks first: chunk 1 must cover the w0/w1 HBM stream, and a
    small final chunk shortens the kernel drain tail.
    """
    if C <= 256:
        return [C]
    rem = C % 256
    n = C // 256
    if rem == 0:
        return [256] * n
    if rem >= 128:
        return [256] * n + [rem]
    a = rem + 256
    return [256] * (n - 1) + [a - a // 2, a // 2]


def _build_bass(C: int, repeat: int = 1):
    """Build the single-core SPMD Bass program for capacity C."""
    import concourse.bacc as bacc
    import concourse.mybir as mybir
    from concourse import tile

    fp32 = mybir.dt.float32
    bf16 = mybir.dt.bfloat16
    f8 = mybir.dt.float8e4
    AF = mybir.ActivationFunctionType
    ALU = mybir.AluOpType
    DR = mybir.MatmulPerfMode.DoubleRow

    chunks = _chunk_plan(C)
    starts = [sum(chunks[:i]) for i in range(len(chunks))]

    nc = bacc.Bacc("TRN2", target_bir_lowering=False)
    # one DRAM tensor per token chunk so every chunk DMA is fully contiguous
    xt_ds = [
        nc.dram_tensor(f"xt{i}", [P, 2, KD, w], f8, kind="ExternalInput")
        for i, w in enumerate(chunks)
    ]
    w0_d = nc.dram_tensor("w0t", [P, KH, KD, 2, P], f8, kind="ExternalInput")
    w1_d = nc.dram_tensor("w1t", [P, KH, KD, 2, P], f8, kind="ExternalInput")
    # w2 is d-tile-major so it can stream just-in-time for chunk-1 stage 2
    w2_d = nc.dram_tensor("w2t", [P, KD, KH, 2, P], f8, kind="ExternalInput")
    b0_d = nc.dram_tensor("b0t", [P, KH], fp32, kind="ExternalInput")
    b1_d = nc.dram_tensor("b1t", [P, KH], fp32, kind="ExternalInput")
    # out is produced transposed: out_t[p, k, c] = ffn_out[c, k*128+p]*SW*SACT
    out_d = nc.dram_tensor("out", [P, KD, C], fp32, kind="ExternalOutput")

    with tile.TileContext(nc) as tc:
        with (
            tc.tile_pool(name="wconst", bufs=1) as wpool,
            tc.tile_pool(name="xtp", bufs=3) as xpool,
            tc.tile_pool(name="act", bufs=3) as apool,
            tc.tile_pool(name="sil", bufs=4) as spool,
            tc.tile_pool(name="tmp", bufs=4) as tpool,
            tc.tile_pool(name="osb", bufs=10) as opool,
            tc.tile_pool(name="ps0", bufs=3, space="PSUM") as pp0,
            tc.tile_pool(name="ps1", bufs=2, space="PSUM") as pp1,
            tc.tile_pool(name="pso", bufs=3, space="PSUM") as ppo,
        ):
            w0_sb = wpool.tile([P, KH, KD, 2, P], f8, tag="w0")
            w1_sb = wpool.tile([P, KH, KD, 2, P], f8, tag="w1")
            w2_sb = wpool.tile([P, KD, KH, 2, P], f8, tag="w2")
            b0_sb = wpool.tile([P, KH], fp32, tag="b0")
            b1_sb = wpool.tile([P, KH], fp32, tag="b1")
            # Warm the PE (p-state ramp) with dummy matmuls on a zeroed tile
            # while the first weight/token DMAs are in flight.
            z_sb = wpool.tile([P, P], bf16, tag="warmz")
            nc.vector.memset(z_sb[:], 0.0)
            zp = ppo.tile([P, P], fp32, tag="pso", name="warmp")
            n_warm = 4 if C >= 768 else 12
            for _ in range(n_warm):
                nc.tensor.matmul(zp[:], z_sb[:], z_sb[:], start=True, stop=True)

            # DMA schedule, ordered by consumption deadline: per-h-tile w1/w0
            # pieces feed the interleaved stage-1 of chunks 1+2; w2 d-pieces
            # feed chunk-1 stage 2; later token chunks slot in between.
            xt_tiles = [
                xpool.tile([P, 2, KD, w], f8, tag="xt", name=f"xtt{i}")
                for i, w in enumerate(chunks)
            ]
            nc.sync.dma_start(w1_sb[:, 0:1], w1_d[:, 0:1])
            nc.sync.dma_start(xt_tiles[0][:, 1:2], xt_ds[0][:, 1:2])
            nc.sync.dma_start(xt_tiles[0][:, 0:1], xt_ds[0][:, 0:1])
            nc.sync.dma_start(w0_sb[:, 0:1], w0_d[:, 0:1])
            for ht in range(1, KH):
                nc.sync.dma_start(w1_sb[:, ht:ht + 1], w1_d[:, ht:ht + 1])
                nc.sync.dma_start(w0_sb[:, ht:ht + 1], w0_d[:, ht:ht + 1])
                if ht == 1:
                    nc.sync.dma_start(b1_sb[:], b1_d[:])
                    nc.sync.dma_start(b0_sb[:], b0_d[:])
                if ht == 2 and len(chunks) > 1:
                    nc.sync.dma_start(xt_tiles[1][:], xt_ds[1][:])
            for dk in range(KD):
                nc.sync.dma_start(w2_sb[:, dk:dk + 1], w2_d[:, dk:dk + 1])
            for ci in range(2, len(chunks)):
                nc.sync.dma_start(xt_tiles[ci][:], xt_ds[ci][:])

            act_tiles: dict = {}

            def s1_htile(ci, ht):
                """Stage-1 h-tile: 24 DR matmuls + silu/split-fp8 act path."""
                tcw = chunks[ci]
                xt_sb = xt_tiles[ci]
                if ht == 0:
                    act_tiles[ci] = apool.tile(
                        [P, KH, 2, tcw], f8, tag="act", name=f"act{ci}"
                    )
                act_sb = act_tiles[ci]
                for which, (w_sb, pp) in enumerate(((w1_sb, pp1), (w0_sb, pp0))):
                    ps = pp.tile(
                        [P, tcw], fp32, tag=f"ps{which}", name=f"ps{which}_t"
                    )
                    # main terms: hi (w slot0) x hi (x slot1), 2 subtiles/DR
                    for g in range(KD // 2):
                        nc.tensor.matmul(
                            ps[:],
                            w_sb[:, ht, 2 * g:2 * g + 2, 0, :],
                            xt_sb[:, 1, 2 * g:2 * g + 2, :],
                            start=(g == 0), stop=False, perf_mode=DR,
                        )
                    # cross terms: (Whi,Wlo) x (xlo,xhi) per subtile
                    for s in range(KD):
                        nc.tensor.matmul(
                            ps[:],
                            w_sb[:, ht, s, :, :],
                            xt_sb[:, :, s, :],
                            start=False, stop=(s == KD - 1), perf_mode=DR,
                        )
                    if which == 0:
                        ps1 = ps
                    else:
                        ps0 = ps
                sil = spool.tile([P, tcw], fp32, tag="sil")
                af = AF.Silu if _ACT_SILU else AF.Tanh
                nc.scalar.activation(
                    sil[:], ps1[:], af, bias=b1_sb[:, ht:ht + 1], scale=B1SC
                )
                # tmp = (ps0 + b0') * sil   (fp32)
                tmp = tpool.tile([P, tcw], fp32, tag="tmp")
                nc.vector.scalar_tensor_tensor(
                    tmp[:], ps0[:], b0_sb[:, ht:ht + 1], sil[:],
                    ALU.add, ALU.mult,
                )
                # act_hi = fp8(tmp * B0SC) on the pool engine
                nc.gpsimd.tensor_scalar_mul(act_sb[:, ht, 1, :], tmp[:], B0SC)
                # act_lo = fp8(tmp * B0SC - act_hi) on the vector engine
                nc.vector.scalar_tensor_tensor(
                    act_sb[:, ht, 0, :], tmp[:], B0SC,
                    act_sb[:, ht, 1, :], ALU.mult, ALU.subtract,
                )

            def s2_dtile(ci, dk):
                """Stage-2 d-tile: 24 DR matmuls into psum, copy out, DMA."""
                tcw = chunks[ci]
                c0 = starts[ci]
                act_sb = act_tiles[ci]
                pso = ppo.tile([P, tcw], fp32, tag="pso")
                for g in range(KH // 2):
                    nc.tensor.matmul(
                        pso[:],
                        w2_sb[:, dk, 2 * g:2 * g + 2, 0, :],
                        act_sb[:, 2 * g:2 * g + 2, 1, :],
                        start=(g == 0), stop=False, perf_mode=DR,
                    )
                for s in range(KH):
                    nc.tensor.matmul(
                        pso[:],
                        w2_sb[:, dk, s, :, :],
                        act_sb[:, s, :, :],
                        start=False, stop=(s == KH - 1), perf_mode=DR,
                    )
                o_sb = opool.tile([P, tcw], fp32, tag="osb")
                nc.vector.tensor_copy(o_sb[:], pso[:])
                nc.sync.dma_start(out_d[:, dk, c0:c0 + tcw], o_sb[:])

            def _body():
                """Software pipeline: stage-1 of chunks 1+2 interleaved per
                h-tile up front (each streamed weight h-tile is consumed
                twice back-to-back, halving the required HBM rate), then each
                chunk's stage-2 interleaved with the stage-1 of the chunk two
                ahead. Every stage-2 thus starts long after its own act tiles
                settled."""
                n = len(chunks)
                if n == 1:
                    for ht in range(KH):
                        s1_htile(0, ht)
                else:
                    s1_htile(0, 0)
                    s1_htile(0, 1)
                    for ht in range(2, KH):
                        s1_htile(0, ht)
                        s1_htile(1, ht - 2)
                    s1_htile(1, KH - 2)
                    s1_htile(1, KH - 1)
                for ci in range(n):
                    for dk in range(KD):
                        s2_dtile(ci, dk)
                        if ci + 2 < n:
                            s1_htile(ci + 2, 2 * dk)
                            s1_htile(ci + 2, 2 * dk + 1)

            if repeat == 1:
                _body()
            else:
                with tc.For_i(0, repeat, 1):
                    _body()
    nc.compile()
    return nc


def _get_bass(C: int, repeat: int = 1):
    key = (C, repeat)
    if key not in _build_cache:
        _build_cache[key] = _build_bass(C, repeat)
    return _build_cache[key]


_runner_cache: dict = {}


def _get_runner(C: int, repeat: int = 1):
    """Compile the SPMD program once and return a reusable launcher."""
    key = (C, repeat)
    if key in _runner_cache:
        return _runner_cache[key]

    import jax
    from jax.experimental.shard_map import shard_map
    from jax.sharding import Mesh, PartitionSpec
    import concourse.mybir as mybir
    from concourse import bass2jax

    nc = _get_bass(C, repeat)
    bass2jax.install_neuronx_cc_hook()
    partition_name = nc.partition_id_tensor.name if nc.partition_id_tensor else None

    in_names: list = []
    out_names: list = []
    out_avals: list = []
    out_shapes: list = []
    for alloc in nc.m.functions[0].allocations:
        if not isinstance(alloc, mybir.MemoryLocationSet):
            continue
        name = alloc.memorylocations[0].name
        if alloc.kind == "ExternalInput":
            if name != partition_name:
                in_names.append(name)
        elif alloc.kind == "ExternalOutput":
            shape = tuple(alloc.tensor_shape)
            dtype = mybir.dt.np(alloc.dtype)
            out_names.append(name)
            out_avals.append(jax.core.ShapedArray(shape, dtype))
            out_shapes.append((shape, dtype))
    n_params = len(in_names)
    all_names = list(in_names) + list(out_names)
    if partition_name is not None:
        all_names.append(partition_name)
    donate = tuple(range(n_params, n_params + len(out_names)))

    def _body(*args):
        operands = list(args)
        if partition_name is not None:
            operands.append(bass2jax.partition_id_tensor())
        outs = bass2jax._bass_exec_p.bind(
            *operands,
            out_avals=tuple(out_avals),
            in_names=tuple(all_names),
            out_names=tuple(out_names),
            lowering_input_output_aliases=(),
            sim_require_finite=True,
            sim_require_nnan=True,
            nc=nc,
        )
        return tuple(outs)

    devices = jax.devices()[:NCORES]
    assert len(devices) == NCORES
    mesh = Mesh(np.asarray(devices), ("core",))
    in_specs = (PartitionSpec("core"),) * (n_params + len(out_names))
    out_specs = (PartitionSpec("core"),) * len(out_names)
    sharded = jax.jit(
        shard_map(
            _body, mesh=mesh, in_specs=in_specs, out_specs=out_specs, check_rep=False
        ),
        donate_argnums=donate,
        keep_unused=True,
    )

    def run(in_maps):
        concat_in = [
            np.concatenate([np.asarray(in_maps[c][nm]) for c in range(NCORES)], axis=0)
            for nm in in_names
        ]
        concat_zeros = [
            np.zeros((NCORES * s[0], *s[1:]), dt) for s, dt in out_shapes
        ]
        out_arrs = sharded(*concat_in, *concat_zeros)
        return [
            {
                nm: np.asarray(out_arrs[i]).reshape(NCORES, *out_shapes[i][0])[c]
                for i, nm in enumerate(out_names)
            }
            for c in range(NCORES)
        ]

    _runner_cache[key] = run
    return run


def _route(x2d: np.ndarray, gate_w: np.ndarray, gate_b: np.ndarray):
    """Top-2 routing on the host (f64 logits for stable ordering)."""
    lg = x2d.astype(np.float64) @ gate_w.astype(np.float64).T
    lg += gate_b.astype(np.float64)
    order = np.argsort(-lg, axis=1, kind="stable")
    ti = order[:, :TOPK]
    tv = np.take_along_axis(lg, ti, axis=1)
    m = tv.max(axis=1, keepdims=True)
    ew = np.exp(tv - m)
    wk = ew / ew.sum(axis=1, keepdims=True)
    return ti, wk


def _split8(a: np.ndarray, s: float):
    """a*s as a sum of two e4m3 tensors (hi + lo, lo at the same scale)."""
    asf = (a * s).astype(np.float32)
    hi = asf.astype(F8)
    lo = (asf - hi.astype(np.float32)).astype(F8)
    return hi, lo


def _pack_x(x2d: np.ndarray, C: int):
    """[n, D] tokens -> [128, 2, KD, C] (slot0=lo, slot1=hi)."""
    n = x2d.shape[0]
    hi, lo = _split8(x2d, SX)
    out = np.zeros((P, 2, KD, C), dtype=F8)
    ht = hi.T.reshape(KD, P, n)
    lt = lo.T.reshape(KD, P, n)
    out[:, 1, :, :n] = ht.transpose(1, 0, 2)
    out[:, 0, :, :n] = lt.transpose(1, 0, 2)
    return out


def _pack_w01(w: np.ndarray):
    """[H, D] weight -> [128, KH, KD, 2, 128] (slot0=hi, slot1=lo)."""
    hi, lo = _split8(w, SW)
    r = lambda a: a.reshape(KH, P, KD, P).transpose(3, 0, 2, 1)
    return np.ascontiguousarray(np.stack([r(hi), r(lo)], axis=3))


def _pack_w2(w: np.ndarray):
    """[D, H] weight -> [128, KD, KH, 2, 128] (slot0=hi, slot1=lo).

    w2t[p, dk, ht, slot, dcol] = (hi|lo)[dk*128+dcol, ht*128+p]
    """
    hi, lo = _split8(w, SW)
    r = lambda a: a.reshape(KD, P, KH, P).transpose(3, 0, 2, 1)
    return np.ascontiguousarray(np.stack([r(hi), r(lo)], axis=3))


def _prepare(x, gate_w, gate_b, w0, b0, w1, b1, w2, b2):
    """Host-side routing + per-core input packing. Returns (in_maps, meta)."""
    x = np.asarray(x)
    gate_w = np.asarray(gate_w, dtype=np.float32)
    gate_b = np.asarray(gate_b, dtype=np.float32)
    w0 = np.asarray(w0, dtype=np.float32)
    b0 = np.asarray(b0, dtype=np.float32)
    w1 = np.asarray(w1, dtype=np.float32)
    b1 = np.asarray(b1, dtype=np.float32)
    w2 = np.asarray(w2, dtype=np.float32)
    b2 = np.asarray(b2, dtype=np.float32)

    Bn, Sq, Dv = x.shape
    T = Bn * Sq
    x2d = np.ascontiguousarray(x.reshape(T, Dv)).astype(np.float32, copy=False)

    ti, wk = _route(x2d, gate_w, gate_b)

    idxs, wgts = [], []
    for e in range(E):
        sel = [np.nonzero(ti[:, k] == e)[0] for k in range(TOPK)]
        idxs.append(np.concatenate(sel))
        wgts.append(np.concatenate([wk[s, k] for k, s in enumerate(sel)]))

    maxc = max(len(i) for i in idxs)
    C = max(P, maxc)

    chunks = _chunk_plan(C)
    in_maps = []
    for e in range(E):
        xt_full = _pack_x(x2d[idxs[e]], C)
        xt_parts = {}
        cpos = 0
        for i, w in enumerate(chunks):
            xt_parts[f"xt{i}"] = np.ascontiguousarray(
                xt_full[:, :, :, cpos:cpos + w]
            )
            cpos += w
        in_maps.append(
            {
                **xt_parts,
                "w0t": _pack_w01(w0[e]),
                "w1t": _pack_w01(w1[e]),
                "w2t": _pack_w2(w2[e]),
                # b0 enters as (ps0 + b0*SX*SW) on the device
                "b0t": np.ascontiguousarray(
                    (b0[e] * (SX * SW)).reshape(KH, P).T.astype(np.float32)
                ),
                "b1t": np.ascontiguousarray(b1[e].reshape(KH, P).T),
            }
        )
    meta = (Bn, Sq, Dv, T, C, idxs, wgts, b2)
    return in_maps, meta


def _combine(results, meta):
    Bn, Sq, Dv, T, C, idxs, wgts, b2 = meta
    desc = 1.0 / (SW * SACT)  # stage-2 psum carries SW*SACT
    out = np.zeros((T, Dv), dtype=np.float32)
    for e in range(E):
        n = len(idxs[e])
        # out_t [128, KD, C] -> [C, D] with d = k*128 + p
        ot = np.asarray(results[e]["out"])
        o = ot.transpose(2, 1, 0).reshape(C, Dv)[:n]
        out[idxs[e]] += wgts[e][:, None].astype(np.float32) * (
            o * desc + b2[e][None, :]
        )
    return out.reshape(Bn, Sq, Dv)


def kernel(x, gate_w, gate_b, w0, b0, w1, b1, w2, b2):
    in_maps, meta = _prepare(x, gate_w, gate_b, w0, b0, w1, b1, w2, b2)
    C = meta[4]
    run = _get_runner(C)
    try:
        results = run(in_maps)
    except Exception:
        # transient device hiccups happen on the tunneled cores; retry once
        import time as _time

        _time.sleep(2.0)
        try:
            results = run(in_maps)
        except Exception:
            # last resort: rebuild the PJRT client + executable from scratch
            import jax

            _runner_cache.clear()
            try:
                jax.clear_caches()
                jax.extend.backend.clear_backends()
            except Exception:
                pass
            _time.sleep(5.0)
            results = _get_runner(C)(in_maps)
    return _combine(results, meta)

